# revision 25
# baseline (speedup 1.0000x reference)
"""NerfMLP TRN2 kernel: 8-way data-parallel over tokens, fused 8-layer MLP on-chip.

Layout: feature-major ("transposed") activations [features(partitions), tokens(free)].
Positional encoding computed on-device: range-reduce arg to [-pi, pi] via
fp32 magic-constant round-to-nearest, then ACT Sin (one table set:
silu_and_others holds sin+relu+tanh).

Matmuls in fp16 (1 cyc/row on PE), accumulation fp32 in PSUM.
Bias+ReLU fused into single ACT/DVE ops reading PSUM, split across both
engines to stay under the PE roofline.
"""
import sys
sys.path.insert(0, "/opt/trn_rl_repo")
import numpy as np
import concourse.bass as bass
import concourse.tile as tile
from concourse import bacc, mybir
from concourse.bass_utils import run_bass_kernel_spmd

dt = mybir.dt
AF = mybir.ActivationFunctionType
ALU = mybir.AluOpType

# problem constants (hardcoded per contract)
B, N = 4, 262144
NUM_FREQ = 10
HIDDEN = 256
ENC_DIM = 40
OUT_DIM = 3
N_CORES = 8
TOK = B * N                  # 1048576
TPC = TOK // N_CORES         # 131072 tokens per core
TT = 1024                    # tokens per tile
NT = TPC // TT               # 128 tiles
NB = TT // 512               # matmul N-subtiles per tile
MAGIC = float(np.float32(1.5 * 2.0 ** 23))
TWO_PI = float(2.0 * np.pi)

# packed weight sbuf column layout (fp16): [Win_m0 | Win_m1 | Whid(l,k,m) x24 | Wout_k0 | Wout_k1]
WIN_COL = [0, 128]
def HID_COL(l, k, m):
    return 256 + ((l * 2 + k) * 2 + m) * 128
WOUT_COL = [256 + 3072, 256 + 3072 + 3]
W_COLS = 256 + 3072 + 6     # 3334

# bias sbuf column layout (fp32): 14 cols L(l)m + b_out + enc scale + enc bias
def BIAS_COL(l, m):
    return l * 2 + m
BOUT_COL = 14
ENC_SCALE_COL = 15
ENC_BIAS_COL = 16
B_COLS = 17

# which engine applies bias+relu for (layer, m): balance ACT vs DVE so that
# with sin+tanh on ACT both engines stay under the PE roofline
def relu_on_act(l, m):
    return m == 0 or l == 6


def _pin_act_table_set(keep="silu_and_others"):
    """Force every activation onto one table set (it holds sin+relu+tanh),
    preserving act_func_set indices, so zero mid-kernel table reloads."""
    import concourse.hw_specs as hw_specs
    orig = hw_specs.get_activation_tables
    import concourse.bacc as bacc_mod

    def patched(arch):
        tabs = orig(arch)
        return {name: (funcs if name == keep else set()) for name, funcs in tabs.items()}

    bacc_mod.get_activation_tables = patched

_NC_CACHE = {}
LAST_RESULTS = None


def _build_nc():
    _pin_act_table_set()
    nc = bacc.Bacc(None, target_bir_lowering=False)

    xT_d = nc.dram_tensor("xT", [2, TPC], dt.float32, kind="ExternalInput")
    w_d = nc.dram_tensor("wts", [128, W_COLS], dt.float16, kind="ExternalInput")
    b_d = nc.dram_tensor("bias", [128, B_COLS], dt.float32, kind="ExternalInput")
    out_d = nc.dram_tensor("out", [OUT_DIM, TPC], dt.float32, kind="ExternalOutput")

    with tile.TileContext(nc) as tc:
        from contextlib import ExitStack
        with ExitStack() as ctx:
            wp = ctx.enter_context(tc.tile_pool(name="wp", bufs=1))
            xp = ctx.enter_context(tc.tile_pool(name="xp", bufs=6))
            ep = ctx.enter_context(tc.tile_pool(name="ep", bufs=5))
            hp = ctx.enter_context(tc.tile_pool(name="hp", bufs=10))
            op = ctx.enter_context(tc.tile_pool(name="op", bufs=2))
            pp = ctx.enter_context(tc.tile_pool(name="pp", bufs=4, space="PSUM"))

            W = wp.tile([128, W_COLS], dt.float16)
            nc.sync.dma_start(out=W, in_=w_d[:, :])
            Bb = wp.tile([128, B_COLS], dt.float32)
            nc.sync.dma_start(out=Bb, in_=b_d[:, :])
            zb = wp.tile([128, 1], dt.float32)
            nc.vector.memset(zb, 0.0)

            def emit_enc(it):
                t0 = it * TT
                xb = xp.tile([ENC_DIM, TT], dt.float32, tag="xb")
                for c in range(2):
                    src = xT_d[c, t0:t0 + TT]
                    src_b = bass.AP(
                        tensor=src.tensor, offset=src.offset,
                        ap=[[0, 20], *[list(p) for p in src.ap]])
                    nc.sync.dma_start(out=xb[c * 20:(c + 1) * 20, :], in_=src_b)

                u = ep.tile([ENC_DIM, TT], dt.float32, tag="u")
                u_i = nc.vector.tensor_scalar(
                    out=u, in0=xb,
                    scalar1=Bb[0:ENC_DIM, ENC_SCALE_COL:ENC_SCALE_COL + 1],
                    scalar2=Bb[0:ENC_DIM, ENC_BIAS_COL:ENC_BIAS_COL + 1],
                    op0=ALU.mult, op1=ALU.add)
                r = ep.tile([ENC_DIM, TT], dt.float32, tag="r")
                nc.vector.tensor_scalar(
                    out=r, in0=u, scalar1=MAGIC, scalar2=-MAGIC,
                    op0=ALU.add, op1=ALU.add)
                f = ep.tile([ENC_DIM, TT], dt.float32, tag="f")
                nc.gpsimd.tensor_tensor(out=f, in0=u, in1=r, op=ALU.subtract)
                enc = ep.tile([ENC_DIM, TT], dt.float16, tag="enc")
                nc.scalar.activation(enc, f, AF.Sin,
                                     bias=zb[0:ENC_DIM, 0:1], scale=TWO_PI)
                return {"enc": enc, "h": {}, "ri": {}, "t0": t0, "u_i": u_i}

            def emit_stage(st, l):
                # m1 emitted first (its psum completes a half-stage early),
                # and k=1 consumed first next stage: the DVE-relu'd half
                # (m1) gets the longer producer->consumer window
                if l == 0:
                    for m in (1, 0):
                        ps = pp.tile([128, TT], dt.float32, tag="ps")
                        wc = WIN_COL[m]
                        for nb in range(NB):
                            nc.tensor.matmul(
                                out=ps[:, nb * 512:(nb + 1) * 512],
                                lhsT=W[0:ENC_DIM, wc:wc + 128],
                                rhs=st["enc"][:, nb * 512:(nb + 1) * 512],
                                start=True, stop=True)
                        st["h"][(0, m)], st["ri"][(0, m)] = _bias_relu(nc, hp, Bb, zb, 0, m, ps)
                elif l <= 6:
                    for m in (1, 0):
                        ps = pp.tile([128, TT], dt.float32, tag="ps")
                        for ki, k in enumerate((1, 0)):
                            wc = HID_COL(l - 1, k, m)
                            for nb in range(NB):
                                nc.tensor.matmul(
                                    out=ps[:, nb * 512:(nb + 1) * 512],
                                    lhsT=W[:, wc:wc + 128],
                                    rhs=st["h"][(l - 1, k)][:, nb * 512:(nb + 1) * 512],
                                    start=(ki == 0), stop=(ki == 1))
                        st["h"][(l, m)], st["ri"][(l, m)] = _bias_relu(nc, hp, Bb, zb, l, m, ps)
                else:
                    pso = pp.tile([OUT_DIM, TT], dt.float32, tag="ps")
                    for ki, k in enumerate((1, 0)):
                        wc = WOUT_COL[k]
                        for nb in range(NB):
                            nc.tensor.matmul(
                                out=pso[:, nb * 512:(nb + 1) * 512],
                                lhsT=W[:, wc:wc + OUT_DIM],
                                rhs=st["h"][(6, k)][:, nb * 512:(nb + 1) * 512],
                                start=(ki == 0), stop=(ki == 1))
                    t1 = op.tile([OUT_DIM, TT], dt.float32, tag="t1")
                    nc.scalar.activation(t1, pso, AF.Tanh,
                                         bias=Bb[0:OUT_DIM, BOUT_COL:BOUT_COL + 1],
                                         scale=1.0)
                    o1 = op.tile([OUT_DIM, TT], dt.float32, tag="o1")
                    nc.vector.tensor_scalar(out=o1, in0=t1, scalar1=0.01,
                                            scalar2=None, op0=ALU.mult)
                    nc.sync.dma_start(out=out_d[:, st["t0"]:st["t0"] + TT], in_=o1)

            # interleave pairs of token tiles so PE never waits on the
            # relu of the layer it just produced (FIFO engine queue);
            # encode one pair ahead so sin is never behind the relu backlog
            from concourse.tile import add_dep_helper
            states = [emit_enc(0), emit_enc(1)]
            for it in range(0, NT, 2):
                nxt = []
                for l in range(8):
                    emit_stage(states[0], l)
                    emit_stage(states[1], l)
                    if l == 2 and it + 2 < NT:
                        nxt = [emit_enc(it + 2), emit_enc(it + 3)]
                states = nxt

    nc.finalize()
    return nc


def _bias_relu(nc, hp, Bb, zb, l, m, ps):
    hh = hp.tile([128, TT], dt.float16, tag="h")
    bias_ap = Bb[:, BIAS_COL(l, m):BIAS_COL(l, m) + 1]
    if relu_on_act(l, m):
        ri = nc.scalar.activation(hh, ps, AF.Relu, bias=bias_ap, scale=1.0)
    else:
        ri = nc.vector.tensor_scalar(out=hh, in0=ps, scalar1=bias_ap,
                                     scalar2=zb[:, 0:1], op0=ALU.add, op1=ALU.max)
    return hh, ri


def _pack_host(W_in, b_in, W_hid, b_hid, W_out, b_out):
    wts = np.zeros((128, W_COLS), np.float16)
    for m in range(2):
        wts[0:ENC_DIM, WIN_COL[m]:WIN_COL[m] + 128] = \
            W_in[:, m * 128:(m + 1) * 128].astype(np.float16)
    for l in range(6):
        for k in range(2):
            for m in range(2):
                wc = HID_COL(l, k, m)
                wts[:, wc:wc + 128] = \
                    W_hid[l, k * 128:(k + 1) * 128, m * 128:(m + 1) * 128].astype(np.float16)
    for k in range(2):
        wc = WOUT_COL[k]
        wts[:, wc:wc + OUT_DIM] = W_out[k * 128:(k + 1) * 128, :].astype(np.float16)

    bia = np.zeros((128, B_COLS), np.float32)
    for m in range(2):
        bia[:, BIAS_COL(0, m)] = b_in[m * 128:(m + 1) * 128]
        for l in range(1, 7):
            bia[:, BIAS_COL(l, m)] = b_hid[l - 1, m * 128:(m + 1) * 128]
    bia[0:OUT_DIM, BOUT_COL] = b_out
    # enc scale/bias rows: f = c*20 + s*10 + k
    for c in range(2):
        for s in range(2):
            for k in range(NUM_FREQ):
                fidx = c * 20 + s * 10 + k
                bia[fidx, ENC_SCALE_COL] = np.float32(2.0 ** k) / np.float32(TWO_PI)
                bia[fidx, ENC_BIAS_COL] = 0.25 if s == 1 else 0.0
    return wts, bia


def kernel(x, W_in, b_in, W_hid, b_hid, W_out, b_out):
    global LAST_RESULTS
    x = np.asarray(x, np.float32)
    wts, bia = _pack_host(
        np.asarray(W_in, np.float32), np.asarray(b_in, np.float32),
        np.asarray(W_hid, np.float32), np.asarray(b_hid, np.float32),
        np.asarray(W_out, np.float32), np.asarray(b_out, np.float32))

    if "nc" not in _NC_CACHE:
        _NC_CACHE["nc"] = _build_nc()
    nc = _NC_CACHE["nc"]

    xf = x.reshape(TOK, 2)
    in_maps = []
    for c in range(N_CORES):
        xs = np.ascontiguousarray(xf[c * TPC:(c + 1) * TPC, :].T)  # [2, TPC]
        in_maps.append({"xT": xs, "wts": wts, "bias": bia})

    import os
    trace = bool(os.environ.get("NERF_TRACE"))
    res = run_bass_kernel_spmd(nc, in_maps, list(range(N_CORES)), trace=trace)
    LAST_RESULTS = res

    out = np.empty((TOK, OUT_DIM), np.float32)
    for c in range(N_CORES):
        out[c * TPC:(c + 1) * TPC, :] = res.results[c]["out"].T
    return out.reshape(B, N, OUT_DIM)
